# revision 18
# baseline (speedup 1.0000x reference)
"""BinnedColorLoss Trainium2 kernel (v3).

loss = -mean_{b,h,w}[ (sum_k logp[b, idx_k, h, w] * wts_k) * w ]
with logp = log_softmax(pred, axis=1), idx/wts/w gathered per-pixel from
313-entry KNN tables via the pixel's bin t = binned_color[b,0,h,w].

Math restructuring (per pixel, t = bin, lse = logsumexp over C):
  sum_k logp[idx_k]*wts_k*w = (sum_k pred[idx_k]*wts_k)*w - lse * (w*sum_k wts_k)
With A[t,c] = w[t]*sum_k wts[t,k]*[idx[t,k]=c] and coef(pix) = w[t]*sum_k wts,
and N = B*H*W:
  loss = ( sum_pix lse(pix)*coef(pix) - sum_pix <A[t(pix)], pred[:,pix]> ) / N

Device strategy (v3; data-parallel over 8 cores, 2 images each):
  - Pixels are SORTED BY BIN on the host (the loss is an order-free mean).
    Each 2048-pixel block then spans <=22 consecutive bins, so the G-term
    becomes: per 16-chunk block, one-hot "segment" DoubleRow matmuls
    S[j, c] += onehot[pix, j] * pred[pix, c]  (j = bin - t0, 64-wide window)
    accumulated in PSUM over 8 chunk-pairs, then ONE scalar_tensor_tensor
    dot against the window's A-rows (streamed from host, 64x320 fp8/blk).
    This removes the 10.5MB dense A-row stream of v2 entirely.
  - lse path: chunks are split between ACT (true exp, fp8 in f16 out) and
    GPSIMD (Schraudolph fast-exp: one tensor_scalar fp8->i16 computing
    round(1024*(x*log2e + 15) + C); the i16 bit pattern IS ~exp(x) when
    bitcast to f16). Both write the same f16 tile; one DVE halving add
    tree per group sums 320 channels -> sume; Ln at piece boundaries.
  - DMA: pred (10.5MB) + onehot B (2.1MB) + A windows (0.33MB) + coef,
    pred transfers alternate between the Sync and Tensor DGE rings.
Host combines the 8 per-core [128, 8] partials: loss = (L - G)/N.
"""

import os
import sys

for _p in ("/opt/trn_rl_repo",):
    if _p not in sys.path:
        sys.path.insert(0, _p)

from contextlib import ExitStack

import numpy as np

import concourse.bacc as bacc
import concourse.bass as bass  # noqa: F401
import concourse.mybir as mybir
from concourse import bass_utils, tile

F32 = mybir.dt.float32
F16 = mybir.dt.float16
I16 = mybir.dt.int16
FP8 = mybir.dt.float8e4

B, C, H, W, K = 16, 313, 128, 128, 5
CP = 320                   # C padded (even tree levels, aligned rows)
NCORES = 8
BPC = B // NCORES          # images per core
PIX = BPC * H * W          # pixels per core (32768)
P = 128                    # pixels per chunk (partition dim)
NCHUNK = PIX // P          # 256
G = 16                     # max chunks per group (tile size)
NTOT = B * H * W           # mean denominator
WIN = 64                   # bin window per 16-chunk block (span <= ~22)
BLKC = 32                  # chunks per PSUM block
NBLK = NCHUNK // BLKC      # 8 PSUM blocks
PAD_VAL = -10.0            # pad: exp()~4.5e-5, fastexp bits ~527 -> ~3e-5
# Schraudolph fast-exp constants: i16 = round(x*SCHS + SCHB); bitcast f16.
LOG2E = 1.4426950408889634
SCHS = 1024.0 * LOG2E
SCHB = 1024.0 * 15 - 60.0
# warm-up/cool-down schedule: small first groups so the first ACT starts
# early, small last groups so the final tree+Ln tail is short
G_LIST = [4, 4, 8] + [16] * 14 + [8, 4, 4]   # chunks per group (sums to 256)
# lse pieces: (first_grp, end_grp, n_chunks, emit_after_grp, out_col).
PIECES = [
    (0, 10, 128, 13, 5),     # chunks [0,128)   ready g9,  Ln after grp 13
    (10, 19, 124, 19, 6),    # chunks [128,252) ready g18, Ln after grp 19
    (19, 20, 4, -1, 0),      # chunks [252,256) final tail (last 4 chunks)
]


def _t_groups():
    return {
        int(x) for x in os.environ.get("KERNEL_T_GROUPS", "").split(",")
        if x
    }


def build_program():
    act_frac = float(os.environ.get("KERNEL_ACT_FRAC", "0.70"))
    act_frac_t = float(os.environ.get("KERNEL_ACT_FRAC_T", "0.55"))
    stt_eng = os.environ.get("KERNEL_STT_ENGINE", "vector")
    gps_tree = {
        int(x) for x in os.environ.get("KERNEL_GPS_TREE", "").split(",") if x
    }
    t_groups = _t_groups()
    dma_rings = int(os.environ.get("KERNEL_DMA_RINGS", "2"))

    nc = bacc.Bacc(
        "TRN2",
        target_bir_lowering=False,
        debug=False,
        enable_asserts=False,
        num_devices=NCORES,
    )
    # Prefer an activation-table set containing BOTH Exp and Ln so the
    # mid-run Ln pieces don't force ~1.3us exp<->ln table re-loads.
    if os.environ.get("KERNEL_TABLE_REORDER", "1") == "1":
        import concourse.hw_specs as hw_specs

        tabs = hw_specs.get_activation_tables(nc.m.arch)
        _E = mybir.ActivationFunctionType.Exp
        _L = mybir.ActivationFunctionType.Ln
        if any(_E in v and _L in v for v in tabs.values()):
            combined = next(k for k, v in tabs.items() if _E in v and _L in v)
            for k, v in tabs.items():
                if k != combined:
                    v.discard(_E)
                    v.discard(_L)

    ntch = sum(G_LIST[g] for g in t_groups)
    pab_d = nc.dram_tensor(
        "pab_t", [P, NCHUNK, CP + WIN], FP8, kind="ExternalInput"
    ).ap()
    predT_d = nc.dram_tensor(
        "predT_t", [P, max(ntch, 1), 3, P], FP8, kind="ExternalInput"
    ).ap()
    ntiles_h = max(sum(G_LIST[g] for g in t_groups) // 12, 1)
    coefT_d = nc.dram_tensor(
        "coefT_t", [P, ntiles_h, 4 * P], F32, kind="ExternalInput"
    ).ap()
    awin_d = nc.dram_tensor("awin_t", [WIN, NBLK, CP], FP8, kind="ExternalInput").ap()
    coef_d = nc.dram_tensor("coef_t", [P, NCHUNK], F32, kind="ExternalInput").ap()
    out_d = nc.dram_tensor("out", [P, 8], F32, kind="ExternalOutput").ap()

    with tile.TileContext(nc) as tc, ExitStack() as ctx, nc.allow_low_precision(
        "f16 exp-sum tree + fp8 G matmuls; validated rel err ~1e-3 << 2e-2 tol"
    ):
        const = ctx.enter_context(tc.tile_pool(name="const", bufs=1))
        predp = ctx.enter_context(tc.tile_pool(name="pred", bufs=6))
        expp = ctx.enter_context(tc.tile_pool(name="exp", bufs=3))
        trp = ctx.enter_context(tc.tile_pool(name="tree", bufs=3))
        accp = ctx.enter_context(tc.tile_pool(name="acc", bufs=1))
        psum = ctx.enter_context(tc.tile_pool(name="psum", bufs=2, space="PSUM"))
        psumT = ctx.enter_context(tc.tile_pool(name="psumT", bufs=3, space="PSUM"))
        lseTp = ctx.enter_context(tc.tile_pool(name="lseT", bufs=2))
        expTp = ctx.enter_context(tc.tile_pool(name="expT", bufs=3))
        predTp = ctx.enter_context(tc.tile_pool(name="predT", bufs=3))

        ngrp = len(G_LIST)
        starts = [sum(G_LIST[:i]) for i in range(ngrp)]

        stt_q = getattr(nc, stt_eng)

        # DMA issue: pred+B per group, two groups ahead of the consumer.
        # pred transfers alternate Sync/Tensor rings; B on Sync.
        tiles = {}

        t_sorted = sorted(t_groups)
        t_off = {g: sum(G_LIST[x] for x in t_sorted[:i])
                 for i, g in enumerate(t_sorted)}
        tilesT = {}

        def issue_grp(g):
            if g >= ngrp or g in tiles:
                return
            c0, gsz = starts[g], G_LIST[g]
            pt = predp.tile([P, G, CP + WIN], FP8, tag="pred", name=f"pred{g}")
            q = nc.sync if (dma_rings == 1 or g % 2 == 0) else nc.gpsimd
            q.dma_start(pt[:, 0:gsz, :], pab_d[:, c0:c0 + gsz, :])
            tiles[g] = pt
            if g in t_groups:
                tt = predTp.tile([P, G, 3, P], FP8, tag="predT", name=f"predT{g}")
                o = t_off[g]
                (nc.gpsimd if dma_rings > 1 else nc.sync).dma_start(
                    tt[:, 0:gsz, :, :], predT_d[:, o:o + gsz, :, :]
                )
                tilesT[g] = tt

        for _g in range(4):
            issue_grp(_g)

        coef_t = const.tile([P, NCHUNK], F32, tag="coef")
        nc.sync.dma_start(coef_t[:], coef_d)
        awin_t = const.tile([WIN, NBLK, CP], FP8, tag="awin")
        nc.sync.dma_start(awin_t[:], awin_d)

        ones_t = const.tile([P, 32], F16, tag="ones")
        nc.gpsimd.memset(ones_t[:], 1.0)
        ntiles = max(ntch // 12, 1)
        coefT_t = const.tile([P, ntiles, 4 * P], F32, tag="coefT")
        nc.sync.dma_start(coefT_t[:], coefT_d)
        gwinT_t = accp.tile([P, ntiles], F32, tag="gwinT")
        sT_tiles = {}
        out_t = accp.tile([P, 8], F32, tag="out")
        nc.gpsimd.memset(out_t[:], 0.0)
        gwin_t = accp.tile([WIN, NBLK], F32, tag="gwin")
        sdot_t = accp.tile([WIN, CP], F16, tag="sdot")

        # per-piece lse state (pieces cover only non-T groups; columns in
        # coef_t are packed in the same non-T group order by the host)
        grp_piece = {}
        piece_col = {}
        piece_nch = {}
        piece_base = {}
        base_acc = 0
        for pi, (g_lo, g_hi, _, _, _) in enumerate(PIECES):
            nch = sum(G_LIST[g] for g in range(g_lo, g_hi) if g not in t_groups)
            piece_nch[pi] = max(nch, 1)
            piece_base[pi] = base_acc
            base_acc += nch
            piece_col[pi] = 0
            for g in range(g_lo, g_hi):
                grp_piece[g] = pi
        psume, plse, pscr = [], [], []
        for pi in range(len(PIECES)):
            nch = piece_nch[pi]
            psume.append(accp.tile([P, nch], F16, tag=f"sume{pi}", name=f"sume{pi}"))
            plse.append(accp.tile([P, nch], F32, tag=f"lse{pi}", name=f"lse{pi}"))
            pscr.append(accp.tile([P, nch], F32, tag=f"pscr{pi}", name=f"pscr{pi}"))

        def emit_lse_piece(pi):
            _, _, _, _, col = PIECES[pi]
            if piece_col[pi] == 0:
                return
            nch = piece_nch[pi]
            p_lo = piece_base[pi]
            nc.scalar.activation(
                plse[pi][:], psume[pi][:], mybir.ActivationFunctionType.Ln
            )
            nc.vector.tensor_mul(
                pscr[pi][:], plse[pi][:], coef_t[:, p_lo:p_lo + nch]
            )
            nc.vector.tensor_reduce(
                out_t[:, col:col + 1],
                pscr[pi][:],
                axis=mybir.AxisListType.X,
                op=mybir.AluOpType.add,
            )

        pending_ln = []

        def flush_t_ln():
            while pending_ln:
                ti = pending_ln.pop(0)
                lseTt = lseTp.tile([96, 4 * P], F32, tag="lseTt", name=f"lseT{ti}")
                scrTt = lseTp.tile([96, 4 * P], F32, tag="scrTt", name=f"scrT{ti}")
                nc.scalar.activation(
                    lseTt[:], sT_tiles.pop(ti)[:, :],
                    mybir.ActivationFunctionType.Ln,
                )
                nc.vector.scalar_tensor_tensor(
                    scrTt[:],
                    lseTt[:],
                    1.0,
                    coefT_t[0:96, ti, :],
                    mybir.AluOpType.mult,
                    mybir.AluOpType.mult,
                    accum_out=gwinT_t[0:96, ti:ti + 1],
                )

        s_ps = [None] * NBLK

        def emit_pair(pair):
            """G-term matmul for chunks (2*pair, 2*pair+1)."""
            blk = (2 * pair) // BLKC
            g = next(gg for gg in range(ngrp)
                     if starts[gg] <= 2 * pair < starts[gg] + G_LIST[gg])
            pt = tiles[g]
            o = 2 * pair - starts[g]
            first = (2 * pair) % BLKC == 0
            last = (2 * pair + 2) % BLKC == 0
            if first:
                s_ps[blk] = psum.tile([WIN, CP], F32, tag="s", name=f"s{blk}")
            nc.tensor.matmul(
                s_ps[blk][:, :],
                pt[:, o:o + 2, CP:CP + WIN],
                pt[:, o:o + 2, 0:CP],
                start=first,
                stop=last,
                perf_mode=mybir.MatmulPerfMode.DoubleRow,
            )
            if last:
                stt_q.scalar_tensor_tensor(
                    sdot_t[:],
                    s_ps[blk][:, :],
                    1.0,
                    awin_t[:, blk, :],
                    mybir.AluOpType.mult,
                    mybir.AluOpType.mult,
                    accum_out=gwin_t[:, blk:blk + 1],
                )

        for g in range(ngrp):
            c0, gsz = starts[g], G_LIST[g]
            issue_grp(g + 4)
            pt = tiles[g]
            a_g = max(1, int(round(
                (act_frac_t if g in t_groups else act_frac) * gsz)))

            if g in t_groups:
                # transposed path: exp in [c, pix] layout, PE ones-matmul
                # sums over c (3 blocks of 128 partitions), PSUM rows drain
                # to psumeT via small DMAs. No DVE tree at all.
                tt = tilesT[g]
                o = t_off[g]
                etT = expTp.tile([P, G, 3, P], F16, tag="expT")
                nc.scalar.activation(
                    etT[:, 0:a_g, :, :], tt[:, 0:a_g, :, :],
                    mybir.ActivationFunctionType.Exp,
                )
                if a_g < gsz:
                    nc.gpsimd.tensor_scalar(
                        etT[:, a_g:gsz, :, :].bitcast(I16),
                        tt[:, a_g:gsz, :, :],
                        SCHS,
                        SCHB,
                        mybir.AluOpType.mult,
                        mybir.AluOpType.add,
                    )
                flush_t_ln()
                for b in range(gsz // 4):
                    tb = o // 4 + b
                    ti, base = tb // 3, 32 * (tb % 3)
                    if tb % 3 == 0:
                        sT_tiles[ti] = psumT.tile(
                            [96, 4 * P], F32, tag="sT", name=f"sT{ti}"
                        )
                    for blk in range(3):
                        nc.tensor.matmul(
                            sT_tiles[ti][base:base + 32, :],
                            ones_t[:],
                            etT[:, 4 * b:4 * b + 4, blk, :],
                            start=blk == 0,
                            stop=blk == 2,
                        )
                    if tb % 3 == 2:
                        pending_ln.append(ti)
            else:
                pi = grp_piece[g]
                s_lo = piece_col[pi]
                piece_col[pi] += gsz

                # lse path: ACT exp for chunks [0:a_g], GPSIMD fastexp for
                # the rest -- both land in the same f16 tile (i16 bitcast)
                et = expp.tile([P, G, CP], F16, tag="exp")
                nc.scalar.activation(
                    et[:, 0:a_g, :], pt[:, 0:a_g, 0:CP],
                    mybir.ActivationFunctionType.Exp,
                )
                if a_g < gsz:
                    nc.gpsimd.tensor_scalar(
                        et[:, a_g:gsz, :].bitcast(I16),
                        pt[:, a_g:gsz, 0:CP],
                        SCHS,
                        SCHB,
                        mybir.AluOpType.mult,
                        mybir.AluOpType.add,
                    )
                flush_t_ln()
                tr = trp.tile([P, G, 160], F16, tag="tree")
                nc.vector.tensor_add(
                    tr[:, 0:gsz, 0:160], et[:, 0:gsz, 0:160],
                    et[:, 0:gsz, 160:320]
                )
                nc.vector.tensor_add(
                    tr[:, 0:gsz, 0:80], tr[:, 0:gsz, 0:80], tr[:, 0:gsz, 80:160]
                )
                nc.vector.tensor_reduce(
                    psume[pi][:, s_lo:s_lo + gsz],
                    tr[:, 0:gsz, 0:80],
                    axis=mybir.AxisListType.X,
                    op=mybir.AluOpType.add,
                )
            for pj, (_, _, _, emit_after, _) in enumerate(PIECES):
                if emit_after == g:
                    emit_lse_piece(pj)

            # G path: all chunk-pairs of this group
            for pr in range(c0 // 2, (c0 + gsz) // 2):
                emit_pair(pr)

            tiles.pop(g)

        # final tail: T lse partials, last lse piece; G window-dot partials
        flush_t_ln()
        if ntch:
            nc.vector.tensor_reduce(
                out_t[0:96, 7:8],
                gwinT_t[0:96, :],
                axis=mybir.AxisListType.X,
                op=mybir.AluOpType.add,
            )
        emit_lse_piece(len(PIECES) - 1)
        nc.vector.tensor_reduce(
            out_t[0:WIN, 1:2],
            gwin_t[:],
            axis=mybir.AxisListType.X,
            op=mybir.AluOpType.add,
        )
        nc.sync.dma_start(out_d, out_t[:])

    nc.compile()
    return nc


def host_inputs(pred, binned_color, knn_idx, knn_weights, weights):
    """Per-core input dicts. pred (B,C,H,W) f32; binned (B,1,H,W) int;
    knn_idx (C,K) int; knn_weights (C,K) f32; weights (C,) f32."""
    import ml_dtypes

    fp8 = ml_dtypes.float8_e4m3

    pred = np.asarray(pred, dtype=np.float32)
    binned = np.asarray(binned_color)
    knn_idx = np.asarray(knn_idx).astype(np.int64)
    knn_w = np.asarray(knn_weights, dtype=np.float32)
    wts = np.asarray(weights, dtype=np.float32)

    # A[t, c] = w[t] * sum_k knn_w[t,k] * [knn_idx[t,k] == c], padded to CP
    a_tab = np.zeros((C, CP), dtype=np.float32)
    rows = np.repeat(np.arange(C), K)
    cols = knn_idx.reshape(-1)
    vals = (wts[:, None] * knn_w).reshape(-1)
    np.add.at(a_tab, (rows, cols), vals)
    a_tab8 = a_tab.astype(fp8)

    coef_full = wts * knn_w.sum(axis=1)          # (C,)

    in_maps = []
    for core in range(NCORES):
        bs = slice(core * BPC, (core + 1) * BPC)
        t = binned[bs, 0].reshape(PIX).astype(np.int64)
        order = np.argsort(t, kind="stable")
        ts = t[order]

        t_groups = _t_groups()
        starts = [sum(G_LIST[:i]) for i in range(len(G_LIST))]
        pm = np.full((PIX, CP), PAD_VAL, dtype=np.float32)
        pm[:, :C] = pred[bs].transpose(0, 2, 3, 1).reshape(PIX, C)[order]

        # per-block bin windows + one-hot B
        t0 = np.repeat(ts.reshape(NBLK, BLKC * P)[:, 0], BLKC * P)
        j = ts - t0
        assert j.min() >= 0 and j.max() < WIN, f"window overflow: {j.max()}"
        pab = np.zeros((PIX, CP + WIN), dtype=fp8)
        pab[:, 0:CP] = pm.astype(fp8)
        pab[np.arange(PIX), CP + j] = fp8(1.0)
        pab_t = np.ascontiguousarray(
            pab.reshape(NCHUNK, P, CP + WIN).transpose(1, 0, 2)
        )

        # A window rows: awin[j, blk, :] = A[t0_blk + j]
        t0_blk = ts.reshape(NBLK, BLKC * P)[:, 0]        # (NBLK,)
        idx = t0_blk[None, :] + np.arange(WIN)[:, None]  # (WIN, NBLK)
        awin = np.zeros((WIN, NBLK, CP), dtype=fp8)
        ok = idx < C
        awin[ok] = a_tab8[idx[ok]]

        # transposed pred for T-groups: [P, ntch, 3, P], channel = 128*b + p
        t_sorted = sorted(t_groups)
        t_chunks = [starts[g] + j for g in t_sorted for j in range(G_LIST[g])]
        ntch = len(t_chunks)
        pmT = np.full((PIX, 3 * P), PAD_VAL, dtype=np.float32)
        pmT[:, :C] = pm[:, :C]
        predT = np.empty((P, max(ntch, 1), 3, P), dtype=fp8)
        coef_sorted = coef_full[ts].reshape(NCHUNK, P)    # [chunk, pix]
        ntiles = max(ntch // 12, 1)
        coefT = np.zeros((P, ntiles, 4 * P), dtype=np.float32)
        for o, c in enumerate(t_chunks):
            blkvals = pmT[c * P:(c + 1) * P, :]           # [pix, 384]
            predT[:, o, :, :] = blkvals.reshape(P, 3, P).transpose(2, 1, 0).astype(fp8)
        # coefT: tile ti row 32*(tb%3) = batch tb's 4 chunks chunk-major;
        # duplicate-sum rows stay zero
        for tb in range(ntch // 4):
            cs = t_chunks[4 * tb:4 * tb + 4]
            coefT[32 * (tb % 3), tb // 3, :] = np.concatenate(
                [coef_sorted[c] for c in cs]
            )
        # coef for the tree path: non-T chunks packed in group order
        nt_chunks = [starts[g] + j for g in range(len(G_LIST)) if g not in t_groups
                     for j in range(G_LIST[g])]
        coef = np.zeros((P, NCHUNK), dtype=np.float32)
        if nt_chunks:
            coef[:, 0:len(nt_chunks)] = coef_sorted[nt_chunks].T

        in_maps.append(
            {
                "pab_t": pab_t,
                "awin_t": awin,
                "coef_t": coef,
                "predT_t": predT,
                "coefT_t": coefT,
            }
        )
    return in_maps


def combine_outputs(core_outs):
    """core_outs: list of [128, 8] f32 arrays -> scalar loss."""
    total = 0.0
    for o in core_outs:
        o = o.astype(np.float64)
        lsec = o[:, 0].sum() + o[:, 5].sum() + o[:, 6].sum() + o[:, 7].sum()
        g = o[0:WIN, 1].sum()
        total += lsec - g
    return np.array(total / NTOT, dtype=np.float32)


_NC_CACHE = None


def kernel(pred, _color, binned_color, knn_idx, knn_weights, weights):
    global _NC_CACHE
    if _NC_CACHE is None:
        _NC_CACHE = build_program()
    nc = _NC_CACHE
    in_maps = host_inputs(pred, binned_color, knn_idx, knn_weights, weights)
    res = bass_utils.run_bass_kernel_spmd(nc, in_maps, core_ids=list(range(NCORES)))
    outs = [res.results[i]["out"] for i in range(NCORES)]
    return combine_outputs(outs)


if __name__ == "__main__":
    import jax
    import reference

    with jax.default_device(jax.devices("cpu")[0]):
        inputs = reference.setup_inputs()
        inputs = {k: np.asarray(jax.device_get(v)) for k, v in inputs.items()}
    got = kernel(**inputs)
    print("kernel loss:", got)
